# revision 5
# baseline (speedup 1.0000x reference)
"""AttnBlock (GroupNorm + single-head spatial self-attention + residual) on 8 TRN2 cores.

Sharding: data-parallel over batch — B=16 images, 2 per NeuronCore. Each core runs
an identical Bass/Tile program over its 2 images; no cross-core communication.

Per-image pipeline (all on one core, C=512 channels, HW=1024 spatial):
  1. GroupNorm(32 groups): per-channel sum/sumsq (DVE/ACT) over the bf16 x
     copy, group-combine via a tiny matmul with a 0/1 group-selector,
     broadcast back via its transpose. rstd = exp(-0.5*ln(var+eps)) on ACT —
     keeps every ACT function in the one natural_log_exp table set (no ~2.7us
     table swaps; the set choice is pinned by narrowing the table map handed
     to the insert_act_table_loads pass).
  2. q,k (C x HW, channel-partitioned) and vT (HW x C, spatial-partitioned)
     via 1x1-conv matmuls against pre-transposed weights.
  3. scores^T[j,i] = sum_c k[c,j] q[c,i]; exp (scale folded into the ACT
     activation) -> P^T; den[i] = sum_j P^T via a 32.0-vector matmul whose
     accumulating MMs are interleaved with the exp evictions.
  4. 1/den via exp(-ln(den)): ACT Ln on the 1-partition den row, ones-matmul
     broadcast of ln(den) to 128 partitions, ACT Exp(scale=-1) — this avoids
     DVE's serial ~5 cyc/elem reciprocal on a single lane. The chain is
     emitted after the first num matmul group so it hides behind matmuls.
  5. num[c,i] = sum_j vT[j,c] P^T[j,i]; proj = woT.T @ num; out = x + bo_eff +
     proj * (1/den), where bo_eff = bo + wo@bv is formed on-device once so the
     vT eviction is a plain PSUM->fp8 copy (softmax normalization and the bv
     shift both commute with the channel-wise output projection).

x is loaded ONCE per image in bf16 and serves GroupNorm stats, the hn affine,
and the final residual add (bf16 residual costs ~1e-3 relative error against
the 2e-2 budget and halves the gating DMA bytes).

The attention internals (q/k/v/scores/attn-weights) run in fp8e4m3 with
DoubleRow matmuls: each MM contracts a PAIR of 128-row k-tiles per pass,
halving tensor-engine streaming time vs bf16. Weights are pre-scaled by 32 on
the host so w*32 ~ N(0,1) sits in e4m3's normal range; the 32x factors cancel
in the softmax (exp scale /32^2) and in the numerator/denominator quotient
(the den ones-vector holds 32.0). The wo projection stays bf16 (NUM in bf16)
so the final eviction keeps its single fused scalar_tensor_tensor.

Matmul groups accumulate into 2-bank [P, 2, 512] PSUM tiles so every eviction
is one [128, 1024] pass (the ~300ns per-op engine overhead is paid half as
often). A warm-up chain of matmuls runs during the initial DMA/stats front so
the PE's HAM clock gate reaches 2.4 GHz before the first real matmul; image
b's qkv is emitted between scores(a) and attnout(a) so image a's den->recipb
chain hides behind matmuls.
"""

import numpy as np
import ml_dtypes
from contextlib import ExitStack

import concourse.bass as bass
import concourse.bacc as bacc
import concourse.tile as tile
import concourse.mybir as mybir
from concourse.bass_utils import run_bass_kernel_spmd

F32 = mybir.dt.float32
AF = mybir.ActivationFunctionType
OP = mybir.AluOpType
AX = mybir.AxisListType
DRM = mybir.MatmulPerfMode.DoubleRow

B, C, H, W = 16, 512, 32, 32
HW = H * W            # 1024
G = 32                # groupnorm groups
CPG = C // G          # 16 channels per group
EPS = 1e-5
NCORES = 8
BPC = B // NCORES     # 2 images per core
P = 128               # SBUF partitions
NCT = C // P          # 4 channel tiles
GPT = P // CPG        # 8 groups per channel tile
NSB = HW // P         # 8 spatial blocks of 128
FC = 512              # matmul moving-dim chunk (one PSUM bank of fp32)
NIC = HW // FC        # 2 chunks over the spatial free dim
WS = 32.0             # fp8 weight pre-scale (w*32 ~ N(0,1))
SM_SCALE = float(C) ** -0.5 / (WS * WS)   # exp scale; q,k each carry a 32x
NWARM = 28            # warm-up matmuls covering the DMA/stats front

DT = mybir.dt.bfloat16          # residual-adjacent dtype (x, NUM, wo)
DT_NP = ml_dtypes.bfloat16
F8 = mybir.dt.float8e4          # attention-internals dtype (DoubleRow matmuls)
F8_NP = ml_dtypes.float8_e4m3

_CACHE: dict = {}


def _pin_act_tables():
    """Narrow the ACT table map so exp/ln/square/identity/copy resolve only to
    natural_log_exp_and_others: the insert_act_table_loads pass then emits ONE
    table load instead of thrashing between exp_and_others and natural_log
    (~2.7us per swap). Set order (and so act_func_set_id) is preserved."""
    if _CACHE.get("tables_pinned"):
        return
    orig = bacc.get_activation_tables
    pinned = {AF.Exp, AF.Ln, AF.Square, AF.Identity, AF.Copy}

    def patched(arch):
        tabs = orig(arch)
        return {
            name: (fns if name == "natural_log_exp_and_others" else (fns - pinned))
            for name, fns in tabs.items()
        }

    bacc.get_activation_tables = patched
    _CACHE["tables_pinned"] = True


def _mm(nc, out, lhsT, rhs, start, stop):
    nc.tensor.matmul(out, lhsT, rhs, start=start, stop=stop)


def _mm8(nc, out, lhsT, rhs, start, stop):
    nc.tensor.matmul(out, lhsT, rhs, start=start, stop=stop, perf_mode=DRM)


def _emit(ctx, tc, io):
    nc = tc.nc

    consts = ctx.enter_context(tc.tile_pool(name="consts", bufs=1))
    pXB = ctx.enter_context(tc.tile_pool(name="pXB", bufs=2))
    pHN = ctx.enter_context(tc.tile_pool(name="pHN", bufs=2))
    pQ = ctx.enter_context(tc.tile_pool(name="pQ", bufs=2))
    pK = ctx.enter_context(tc.tile_pool(name="pK", bufs=2))
    pVT = ctx.enter_context(tc.tile_pool(name="pVT", bufs=2))
    pPT = ctx.enter_context(tc.tile_pool(name="pPT", bufs=2))
    pNUM = ctx.enter_context(tc.tile_pool(name="pNUM", bufs=2))
    pOUT = ctx.enter_context(tc.tile_pool(name="pOUT", bufs=2))
    pS = ctx.enter_context(tc.tile_pool(name="pS", bufs=2))
    # 2-bank matmul tiles: [P, NIC, FC] fp32, 3 in flight + one aux ring
    pmm = ctx.enter_context(tc.tile_pool(name="pmm", bufs=3, space="PSUM"))
    paux = ctx.enter_context(tc.tile_pool(name="paux", bufs=1, space="PSUM"))

    # ---- image 0's x (bf16) first: it gates the whole pipeline (stats -> hn
    # -> qkv). Four descriptors alternate over both HWDGE queues.
    def emit_loadx(i, pool):
        XB = pool.tile([P, NCT, HW], DT, name=f"XB{i}", tag="XB")
        for ct in range(NCT):
            (nc.sync if ct % 2 == 0 else nc.scalar).dma_start(
                XB[:, ct, :], io["x"][i, ct * P:(ct + 1) * P, :])
        return XB

    XB_0 = emit_loadx(0, pXB)

    def load_const(name, shape, dtype=F32, q=None):
        t = consts.tile(list(shape), dtype, name=f"c_{name}")
        (q or nc.sync).dma_start(t[:], io[name][:])
        return t

    # all (P, *) vectors packed into ONE DMA — each dma_start costs ~600ns of
    # sync-engine descriptor time that would otherwise delay the weight loads
    cvec = load_const("cvec", (P, 5 * NCT + GPT))
    bq_sb = cvec[:, 0 * NCT:1 * NCT]
    bk_sb = cvec[:, 1 * NCT:2 * NCT]
    bo_sb = cvec[:, 2 * NCT:3 * NCT]
    gs_sb = cvec[:, 3 * NCT:4 * NCT]
    gb_sb = cvec[:, 4 * NCT:5 * NCT]
    gsel = cvec[:, 5 * NCT:5 * NCT + GPT]
    gselT = load_const("gselT", (GPT, P))
    bvcol = load_const("bvcol", (P, NCT), DT)

    # ---- weights (loaded once, shared by both images), one packed DMA per
    # matrix, balanced across the two queues: q/k on sync, v/o on scalar.
    # q/k/v weights are fp8 (x32) with [P, ct, c_out] layout so a
    # [:, ct:ct+2, :] slice is a DoubleRow stationary operand; wo stays bf16.
    w_sb = {}
    for wname, q in (("wqt", nc.sync), ("wkt", nc.sync), ("wvt", nc.scalar)):
        t = consts.tile([P, NCT, C], F8, name=f"{wname}_p")
        q.dma_start(t[:, :, :], io[wname][:])
        w_sb[wname] = t
    wot_sb = consts.tile([P, NCT, C], DT, name="wot_p")
    nc.scalar.dma_start(wot_sb[:, :, :], io["wot"][:])

    ones_col8 = consts.tile([P, 2, 16], F8, name="ones_col8")
    nc.vector.memset(ones_col8[:], WS)   # 32.0: cancels the 32x carried by VT
    ones_row = consts.tile([1, P], DT, name="ones_row")
    nc.vector.memset(ones_row[:], 1.0)
    zb = consts.tile([P, 1], F32, name="zb")
    nc.vector.memset(zb[:], 0.0)
    epsb = consts.tile([GPT, 1], F32, name="epsb")
    nc.vector.memset(epsb[:], EPS)

    # ---- PE warm-up: a serial chain of matmuls spanning the DMA/stats front
    # keeps the HAM activity monitor busy so the clock gate opens to 2.4 GHz
    # (~3.4us in) and STAYS open until the first real matmul. Rotates through
    # the pmm ring so it costs no extra PSUM bank.
    warm8 = consts.tile([P, FC], F8, name="warm8")
    nc.vector.memset(warm8[:], 0.0)
    for w in range(NWARM):
        wp = pmm.tile([1, FC], F32, name=f"warm{w}", tag="mm")
        _mm(nc, wp[:], ones_col8[:, 0, 0:1], warm8[:], start=True, stop=True)

    # ---- per-image emission ----
    def new_img(i):
        return {"i": i}

    def emit_load(im):
        i = im["i"]
        im["XB"] = XB_0 if i == 0 else emit_loadx(i, pXB)

    def emit_stats(im):
        i = im["i"]
        XB = im["XB"]
        stats = pS.tile([P, 2 * NCT], F32, name=f"stats{i}", tag="stats")
        scratch = pS.tile([P, HW], DT, name=f"scr{i}", tag="scratch")
        for ct in range(NCT):
            nc.vector.tensor_reduce(stats[:, ct:ct + 1], XB[:, ct, :], AX.X, OP.add)
            nc.scalar.activation(scratch[:], XB[:, ct, :], AF.Square, bias=zb[:],
                                 accum_out=stats[:, NCT + ct:NCT + ct + 1])
        im["stats"] = stats

    def emit_norm(im):
        i = im["i"]
        XB, stats = im["XB"], im["stats"]
        with nc.named_scope(f"norm{i}"):
            gst = paux.tile([GPT, 2 * NCT], F32, name=f"gst{i}", tag="aux")
            _mm(nc, gst[:], gsel[:], stats[:], start=True, stop=True)
            gm = pS.tile([GPT, 2 * NCT], F32, name=f"gm{i}", tag="gm")
            nc.vector.tensor_scalar_mul(gm[:], gst[:], 1.0 / (CPG * HW))
            sq = pS.tile([GPT, NCT], F32, name=f"sq{i}", tag="sq")
            nc.vector.tensor_mul(sq[:], gm[:, 0:NCT], gm[:, 0:NCT])
            var = pS.tile([GPT, NCT], F32, name=f"var{i}", tag="var")
            nc.vector.tensor_sub(var[:], gm[:, NCT:], sq[:])
            # rstd = exp(-0.5*ln(var+eps)) — Ln/Exp live in one ACT table set,
            # unlike Sqrt (whose set swap costs ~2.7us each way)
            lnv = pS.tile([GPT, NCT], F32, name=f"lnv{i}", tag="lnv")
            nc.scalar.activation(lnv[:], var[:], AF.Ln, bias=epsb[:])
            gmr = pS.tile([GPT, 2 * NCT], F32, name=f"gmr{i}", tag="gmr")
            nc.vector.tensor_copy(gmr[:, 0:NCT], gm[:, 0:NCT])
            nc.scalar.activation(gmr[:, NCT:], lnv[:], AF.Exp, bias=zb[0:GPT, :],
                                 scale=-0.5)
            pmr = paux.tile([P, 2 * NCT], F32, name=f"pmr{i}", tag="aux")
            _mm(nc, pmr[:], gselT[:], gmr[:], start=True, stop=True)
            mr = pS.tile([P, 2 * NCT], F32, name=f"mr{i}", tag="mr")
            nc.vector.tensor_copy(mr[:], pmr[:])
            # a = rstd*scale (cols NCT..), b = gn_bias - mean*a (cols 0..NCT)
            ab = pS.tile([P, 2 * NCT], F32, name=f"ab{i}", tag="ab")
            tb = pS.tile([P, NCT], F32, name=f"tb{i}", tag="tb")
            for ct in range(NCT):
                a_col = ab[:, NCT + ct:NCT + ct + 1]
                nc.vector.tensor_mul(a_col, mr[:, NCT + ct:NCT + ct + 1], gs_sb[:, ct:ct + 1])
                nc.vector.tensor_mul(tb[:, ct:ct + 1], mr[:, ct:ct + 1], a_col)
                nc.vector.tensor_sub(ab[:, ct:ct + 1], gb_sb[:, ct:ct + 1], tb[:, ct:ct + 1])
            HN = pHN.tile([P, NCT, HW], F8, name=f"HN{i}", tag="HN")
            for ct in range(NCT):
                nc.vector.tensor_scalar(HN[:, ct, :], XB[:, ct, :],
                                        ab[:, NCT + ct:NCT + ct + 1], ab[:, ct:ct + 1],
                                        OP.mult, OP.add)
            im["HN"] = HN

    def emit_boeff():
        # bo_eff = bo + wo@bv (both commute past the attention average), formed
        # once so the vT eviction needs no bias add. Emitted after norm(b) so
        # its aux-ring slots never gate the groupnorm matmuls.
        boeff = consts.tile([P, NCT], F32, name="boeff")
        for ob in range(NCT):
            ps = paux.tile([P, 1], F32, name=f"wobv{ob}", tag="aux")
            for ct in range(NCT):
                _mm(nc, ps[:], wot_sb[:, ct, ob * P:(ob + 1) * P], bvcol[:, ct:ct + 1],
                    start=(ct == 0), stop=(ct == NCT - 1))
            nc.vector.tensor_add(boeff[:, ob:ob + 1], bo_sb[:, ob:ob + 1], ps[:])
        return boeff

    def emit_qkv(im):
        i = im["i"]
        HN = im["HN"]
        with nc.named_scope(f"qkv{i}"):
            Q = pQ.tile([P, NCT, HW], F8, name=f"Q{i}", tag="Q")
            K = pK.tile([P, NCT, HW], F8, name=f"K{i}", tag="K")
            for wname, bias_sb, OT, on_act in (("wqt", bq_sb, Q, True),
                                               ("wkt", bk_sb, K, False)):
                for ob in range(NCT):
                    ps = pmm.tile([P, NIC, FC], F32, name=f"{wname}ps{i}_{ob}", tag="mm")
                    for ct in range(0, NCT, 2):
                        lhs = w_sb[wname][:, ct:ct + 2, ob * P:(ob + 1) * P]
                        for ic in range(NIC):
                            _mm8(nc, ps[:, ic, :], lhs, HN[:, ct:ct + 2, ic * FC:(ic + 1) * FC],
                                 start=(ct == 0), stop=(ct == NCT - 2))
                    # one [128,1024] eviction per ob; Q on ACT, K on DVE to
                    # balance the two engines' load
                    if on_act:
                        nc.scalar.add(OT[:, ob, :], ps[:], bias_sb[:, ob:ob + 1])
                    else:
                        nc.vector.tensor_scalar_add(OT[:, ob, :], ps[:],
                                                    bias_sb[:, ob:ob + 1])
            VT = pVT.tile([P, NSB, C], F8, name=f"VT{i}", tag="VT")
            for sb in range(0, NSB, 2):
                ps = pmm.tile([P, 2, C], F32, name=f"vtps{i}_{sb}", tag="mm")
                for k in range(2):
                    for ct in range(0, NCT, 2):
                        _mm8(nc, ps[:, k, :], HN[:, ct:ct + 2, (sb + k) * P:(sb + k + 1) * P],
                             w_sb["wvt"][:, ct:ct + 2, 0:C],
                             start=(ct == 0), stop=(ct == NCT - 2))
                nc.vector.tensor_copy(VT[:, sb:sb + 2, :], ps[:])
            im["Q"], im["K"], im["VT"] = Q, K, VT

    def emit_scores(im):
        i = im["i"]
        Q, K = im["Q"], im["K"]
        with nc.named_scope(f"scores{i}"):
            PT = pPT.tile([P, NSB, HW], F8, name=f"PT{i}", tag="PT")
            # den accumulates across jb pairs; its MMs are emitted inside the
            # jb loop so each lands right after the exp that feeds it
            den = paux.tile([1, NIC, FC], F32, name=f"den{i}", tag="aux")
            for jb in range(NSB):
                ps = pmm.tile([P, NIC, FC], F32, name=f"sps{i}_{jb}", tag="mm")
                for ct in range(0, NCT, 2):
                    lhs = K[:, ct:ct + 2, jb * P:(jb + 1) * P]
                    for ic in range(NIC):
                        _mm8(nc, ps[:, ic, :], lhs, Q[:, ct:ct + 2, ic * FC:(ic + 1) * FC],
                             start=(ct == 0), stop=(ct == NCT - 2))
                nc.scalar.activation(PT[:, jb, :], ps[:], AF.Exp, bias=zb[:],
                                     scale=SM_SCALE)
                if jb % 2 == 1:
                    for ic in range(NIC):
                        _mm8(nc, den[:, ic, :], ones_col8[:, 0:2, 0:1],
                             PT[:, jb - 1:jb + 1, ic * FC:(ic + 1) * FC],
                             start=(jb == 1), stop=(jb == NSB - 1))
            lnden = pS.tile([1, HW], DT, name=f"lnden{i}", tag="lnden")
            nc.scalar.activation(lnden[:], den[:], AF.Ln, bias=zb[0:1, :])
            im["PT"], im["lnden"] = PT, lnden

    def emit_attn_out(im, boeff):
        i = im["i"]
        XB, VT, PT = im["XB"], im["VT"], im["PT"]
        with nc.named_scope(f"attnout{i}"):
            # num = vT.T @ P^T with the 1/den softmax normalization folded into
            # the PSUM eviction (commutes with the channel-wise wo projection);
            # 1/den arrives as exp(-lnden) with the broadcast done by a matmul
            # BETWEEN Ln and Exp so no engine touches 1 lane for long. The
            # chain is emitted after cb0's matmuls, which cover its latency.
            recipb = pS.tile([P, HW], F32, name=f"recipb{i}", tag="recipb")
            NUM = pNUM.tile([P, NCT, HW], DT, name=f"NUM{i}", tag="NUM")
            pss = []
            for cb in range(NCT):
                ps = pmm.tile([P, NIC, FC], F32, name=f"nps{i}_{cb}", tag="mm")
                pss.append(ps)
                for jt in range(0, NSB, 2):
                    lhs = VT[:, jt:jt + 2, cb * P:(cb + 1) * P]
                    for ic in range(NIC):
                        _mm8(nc, ps[:, ic, :], lhs, PT[:, jt:jt + 2, ic * FC:(ic + 1) * FC],
                             start=(jt == 0), stop=(jt == NSB - 2))
                if cb == 0:
                    rb = paux.tile([P, NIC, FC], F32, name=f"rb{i}", tag="aux")
                    for ic in range(NIC):
                        _mm(nc, rb[:, ic, :], ones_row[:],
                            im["lnden"][:, ic * FC:(ic + 1) * FC],
                            start=True, stop=True)
                    nc.scalar.activation(recipb[:], rb[:], AF.Exp, bias=zb[:],
                                         scale=-1.0)
                nc.vector.tensor_mul(NUM[:, cb, :], pss[cb][:], recipb[:])
            # proj + residual (+bo_eff) straight from PSUM, then store
            OUTT = pOUT.tile([P, NCT, HW], F32, name=f"OUT{i}", tag="OUT")
            for ob in range(NCT):
                ps = pmm.tile([P, NIC, FC], F32, name=f"pps{i}_{ob}", tag="mm")
                for ct in range(NCT):
                    lhs = wot_sb[:, ct, ob * P:(ob + 1) * P]
                    for ic in range(NIC):
                        _mm(nc, ps[:, ic, :], lhs, NUM[:, ct, ic * FC:(ic + 1) * FC],
                            start=(ct == 0), stop=(ct == NCT - 1))
                nc.vector.scalar_tensor_tensor(OUTT[:, ob, :], ps[:],
                                               boeff[:, ob:ob + 1], XB[:, ob, :],
                                               OP.add, OP.add)
                (nc.sync if ob % 2 == 0 else nc.scalar).dma_start(
                    io["out"][i, ob * P:(ob + 1) * P, :], OUTT[:, ob, :])

    ims = [new_img(i) for i in range(BPC)]
    a, b = ims
    emit_load(a)
    emit_stats(a)
    emit_load(b)
    emit_stats(b)
    emit_norm(a)
    emit_qkv(a)
    emit_norm(b)
    boeff = emit_boeff()
    emit_scores(a)
    emit_qkv(b)       # between scores(a) and attnout(a): hides a's den chain
    emit_attn_out(a, boeff)
    emit_scores(b)
    emit_attn_out(b, boeff)


def _build():
    if "nc" in _CACHE:
        return _CACHE["nc"]
    _pin_act_tables()
    nc = bacc.Bacc("TRN2", target_bir_lowering=False, debug=False, num_devices=NCORES)
    io = {}
    io["x"] = nc.dram_tensor("x", [BPC, C, HW], DT, kind="ExternalInput").ap()
    for wname in ("wqt", "wkt", "wvt"):
        io[wname] = nc.dram_tensor(wname, [P, NCT, C], F8, kind="ExternalInput").ap()
    io["wot"] = nc.dram_tensor("wot", [P, NCT, C], DT, kind="ExternalInput").ap()
    io["cvec"] = nc.dram_tensor("cvec", [P, 5 * NCT + GPT], F32,
                                kind="ExternalInput").ap()
    io["bvcol"] = nc.dram_tensor("bvcol", [P, NCT], DT, kind="ExternalInput").ap()
    io["gselT"] = nc.dram_tensor("gselT", [GPT, P], F32, kind="ExternalInput").ap()
    io["out"] = nc.dram_tensor("out", [BPC, C, HW], F32, kind="ExternalOutput").ap()

    with tile.TileContext(nc) as tc:
        with ExitStack() as ctx:
            _emit(ctx, tc, io)
    nc.compile()
    _CACHE["nc"] = nc
    return nc


def _col_layout(v):
    # (C,) -> (P, NCT): column ct holds channels [ct*128, (ct+1)*128)
    return np.ascontiguousarray(np.asarray(v, np.float32).reshape(NCT, P).T)


def _run(inputs, trace=False, **run_kwargs):
    x = np.ascontiguousarray(np.asarray(inputs["x"], np.float32).reshape(B, C, HW))
    def _wpack(w, scale, npdt):
        # wT (c_in, c_out) -> (P, NCT, C): W[p, ct, j] = wT[ct*128+p, j] * scale
        wt = (np.asarray(w, np.float32).T * scale).astype(npdt)
        return np.ascontiguousarray(wt.reshape(NCT, P, C).transpose(1, 0, 2))

    wdt = {n: _wpack(inputs[s], WS, F8_NP)
           for n, s in (("wqt", "wq"), ("wkt", "wk"), ("wvt", "wv"))}
    wdt["wot"] = _wpack(inputs["wo"], 1.0, DT_NP)
    pidx = np.arange(P)
    gsel = (pidx[:, None] // CPG == np.arange(GPT)[None, :]).astype(np.float32)
    # bq/bk carry the 32x weight scale so Q=32q, K=32k on-device; bv is folded
    # into bo_eff on-device (bo + wo@bv) so vT needs no bias at all
    cvec = np.concatenate([_col_layout(np.asarray(inputs["bq"]) * WS),
                           _col_layout(np.asarray(inputs["bk"]) * WS),
                           _col_layout(inputs["bo"]), _col_layout(inputs["gn_scale"]),
                           _col_layout(inputs["gn_bias"]), gsel], axis=1)
    common = {
        **wdt,
        "cvec": np.ascontiguousarray(cvec),
        "bvcol": np.ascontiguousarray(_col_layout(inputs["bv"]).astype(DT_NP)),
        "gselT": np.ascontiguousarray(gsel.T),
    }
    xb = x.astype(DT_NP)
    in_maps = [{"x": np.ascontiguousarray(xb[m * BPC:(m + 1) * BPC]), **common}
               for m in range(NCORES)]
    nc = _build()
    res = run_bass_kernel_spmd(nc, in_maps, core_ids=list(range(NCORES)),
                               trace=trace, **run_kwargs)
    out = np.concatenate([r["out"] for r in res.results], axis=0)
    return out.reshape(B, C, H, W).astype(np.float32), res


def kernel(**inputs):
    out, _ = _run(inputs)
    return out


# revision 11
# speedup vs baseline: 1.0280x; 1.0280x over previous
"""AttnBlock (GroupNorm + single-head spatial self-attention + residual) on 8 TRN2 cores.

Sharding: data-parallel over batch — B=16 images, 2 per NeuronCore. Each core runs
an identical Bass/Tile program over its 2 images; no cross-core communication.

Per-image pipeline (all on one core, C=512 channels, HW=1024 spatial):
  1. GroupNorm(32 groups): per-channel sum/sumsq (DVE/ACT) over the bf16 x
     copy, group-combine via a tiny matmul with a 0/1 group-selector,
     broadcast back via its transpose. rstd = exp(-0.5*ln(var+eps)) on ACT —
     keeps every ACT function in the one natural_log_exp table set (no ~2.7us
     table swaps; the set choice is pinned by narrowing the table map handed
     to the insert_act_table_loads pass).
  2. q,k (C x HW, channel-partitioned) and vT (HW x C, spatial-partitioned)
     via 1x1-conv matmuls against pre-transposed weights.
  3. scores^T[j,i] = sum_c k[c,j] q[c,i]; exp (scale folded into the ACT
     activation) -> P^T; den[i] = sum_j P^T via a 32.0-vector matmul whose
     accumulating MMs are interleaved with the exp evictions.
  4. 1/den via exp(-ln(den)): ACT Ln on the 1-partition den row, ones-matmul
     broadcast of ln(den) to 128 partitions, ACT Exp(scale=-1) — this avoids
     DVE's serial ~5 cyc/elem reciprocal on a single lane. The chain is
     emitted after the first num matmul group so it hides behind matmuls.
  5. num[c,i] = sum_j vT[j,c] P^T[j,i]; proj = woT.T @ num; out = x + bo_eff +
     proj * (1/den), where bo_eff = bo + wo@bv is formed on-device once so the
     vT eviction is a plain PSUM->fp8 copy (softmax normalization and the bv
     shift both commute with the channel-wise output projection).

x is loaded ONCE per image in bf16 and serves GroupNorm stats, the hn affine,
and the final residual add (bf16 residual costs ~1e-3 relative error against
the 2e-2 budget and halves the gating DMA bytes).

The attention internals (q/k/v/scores/attn-weights) run in fp8e4m3 with
DoubleRow matmuls: each MM contracts a PAIR of 128-row k-tiles per pass,
halving tensor-engine streaming time vs bf16. Weights are pre-scaled by 32 on
the host so w*32 ~ N(0,1) sits in e4m3's normal range; the 32x factors cancel
in the softmax (exp scale /32^2) and in the numerator/denominator quotient
(the den ones-vector holds 32.0). The wo projection stays bf16 (NUM in bf16)
so the final eviction keeps its single fused scalar_tensor_tensor.

Matmul groups accumulate into 2-bank [P, 2, 512] PSUM tiles so every eviction
is one [128, 1024] pass (the ~300ns per-op engine overhead is paid half as
often). A warm-up chain of matmuls runs during the initial DMA/stats front so
the PE's HAM clock gate reaches 2.4 GHz before the first real matmul; image
b's qkv is emitted between scores(a) and attnout(a) so image a's den->recipb
chain hides behind matmuls.
"""

import numpy as np
import ml_dtypes
from contextlib import ExitStack

import concourse.bass as bass
import concourse.bacc as bacc
import concourse.tile as tile
import concourse.mybir as mybir
from concourse.bass_utils import run_bass_kernel_spmd

F32 = mybir.dt.float32
AF = mybir.ActivationFunctionType
OP = mybir.AluOpType
AX = mybir.AxisListType
DRM = mybir.MatmulPerfMode.DoubleRow

B, C, H, W = 16, 512, 32, 32
HW = H * W            # 1024
G = 32                # groupnorm groups
CPG = C // G          # 16 channels per group
EPS = 1e-5
NCORES = 8
BPC = B // NCORES     # 2 images per core
P = 128               # SBUF partitions
NCT = C // P          # 4 channel tiles
GPT = P // CPG        # 8 groups per channel tile
NSB = HW // P         # 8 spatial blocks of 128
FC = 512              # matmul moving-dim chunk (one PSUM bank of fp32)
NIC = HW // FC        # 2 chunks over the spatial free dim
WS = 32.0             # fp8 weight pre-scale (w*32 ~ N(0,1))
SM_SCALE = float(C) ** -0.5 / (WS * WS)   # exp scale; q,k each carry a 32x
NWARM = 24            # warm-up matmuls covering the DMA/stats front

DT = mybir.dt.bfloat16          # residual-adjacent dtype (x, NUM, wo)
DT_NP = ml_dtypes.bfloat16
F8 = mybir.dt.float8e4          # attention-internals dtype (DoubleRow matmuls)
F8_NP = ml_dtypes.float8_e4m3

_CACHE: dict = {}


def _pin_act_tables():
    """Narrow the ACT table map so exp/ln/square/identity/copy resolve only to
    natural_log_exp_and_others: the insert_act_table_loads pass then emits ONE
    table load instead of thrashing between exp_and_others and natural_log
    (~2.7us per swap). Set order (and so act_func_set_id) is preserved."""
    if _CACHE.get("tables_pinned"):
        return
    orig = bacc.get_activation_tables
    pinned = {AF.Exp, AF.Ln, AF.Square, AF.Identity, AF.Copy}

    def patched(arch):
        tabs = orig(arch)
        return {
            name: (fns if name == "natural_log_exp_and_others" else (fns - pinned))
            for name, fns in tabs.items()
        }

    bacc.get_activation_tables = patched
    _CACHE["tables_pinned"] = True


def _mm(nc, out, lhsT, rhs, start, stop):
    nc.tensor.matmul(out, lhsT, rhs, start=start, stop=stop)


def _mm8(nc, out, lhsT, rhs, start, stop):
    nc.tensor.matmul(out, lhsT, rhs, start=start, stop=stop, perf_mode=DRM)


def _emit(ctx, tc, io):
    nc = tc.nc

    consts = ctx.enter_context(tc.tile_pool(name="consts", bufs=1))
    pXB = ctx.enter_context(tc.tile_pool(name="pXB", bufs=2))
    pHN = ctx.enter_context(tc.tile_pool(name="pHN", bufs=2))
    pQ = ctx.enter_context(tc.tile_pool(name="pQ", bufs=2))
    pK = ctx.enter_context(tc.tile_pool(name="pK", bufs=2))
    pVT = ctx.enter_context(tc.tile_pool(name="pVT", bufs=2))
    pPT = ctx.enter_context(tc.tile_pool(name="pPT", bufs=2))
    pNUM = ctx.enter_context(tc.tile_pool(name="pNUM", bufs=2))
    pOUT = ctx.enter_context(tc.tile_pool(name="pOUT", bufs=2))
    pS = ctx.enter_context(tc.tile_pool(name="pS", bufs=2))
    # 2-bank matmul tiles: [P, NIC, FC] fp32, 3 in flight + one aux ring
    pmm = ctx.enter_context(tc.tile_pool(name="pmm", bufs=3, space="PSUM"))
    paux = ctx.enter_context(tc.tile_pool(name="paux", bufs=1, space="PSUM"))

    # ---- image 0's x (bf16) first: it gates the whole pipeline (stats -> hn
    # -> qkv). Per-queue DMA sustains only ~55 GB/s, so the four 256KB
    # chunks spread over the THREE DMA-capable queues (sync/scalar/gpsimd).
    qs = [nc.sync, nc.scalar, nc.gpsimd, nc.sync]

    def emit_loadx(i, pool):
        XB = pool.tile([P, NCT, HW], DT, name=f"XB{i}", tag="XB")
        for ct in range(NCT):
            qs[ct].dma_start(XB[:, ct, :], io["x"][i, ct * P:(ct + 1) * P, :])
        return XB

    XB_0 = emit_loadx(0, pXB)

    def load_const(name, shape, dtype=F32, q=None):
        t = consts.tile(list(shape), dtype, name=f"c_{name}")
        (q or nc.sync).dma_start(t[:], io[name][:])
        return t

    # all (P, *) vectors packed into ONE DMA — each dma_start costs ~600ns of
    # sync-engine descriptor time that would otherwise delay the weight loads
    cvec = load_const("cvec", (P, 5 * NCT + GPT))
    bq_sb = cvec[:, 0 * NCT:1 * NCT]
    bk_sb = cvec[:, 1 * NCT:2 * NCT]
    bo_sb = cvec[:, 2 * NCT:3 * NCT]
    gs_sb = cvec[:, 3 * NCT:4 * NCT]
    gb_sb = cvec[:, 4 * NCT:5 * NCT]
    gsel = cvec[:, 5 * NCT:5 * NCT + GPT]
    gselT = load_const("gselT", (GPT, P))
    bvcol = load_const("bvcol", (P, NCT), DT)

    # ---- weights (loaded once, shared by both images), one packed DMA per
    # matrix, one queue each. q/k/v weights are fp8 (x32) with [P, ct, c_out]
    # layout so a [:, ct:ct+2, :] slice is a DoubleRow stationary operand; wo
    # stays bf16.
    w_sb = {}
    for qi, wname in enumerate(("wqt", "wkt", "wvt")):
        t = consts.tile([P, NCT, C], F8, name=f"{wname}_p")
        qs[qi].dma_start(t[:, :, :], io[wname][:])
        w_sb[wname] = t
    wot_sb = consts.tile([P, NCT, C], DT, name="wot_p")
    nc.gpsimd.dma_start(wot_sb[:, :, :], io["wot"][:])

    ones_col8 = consts.tile([P, 2, 16], F8, name="ones_col8")
    nc.vector.memset(ones_col8[:], WS)   # 32.0: cancels the 32x carried by VT
    ones_row = consts.tile([1, P], DT, name="ones_row")
    nc.vector.memset(ones_row[:], 1.0)
    zb = consts.tile([P, 1], F32, name="zb")
    nc.vector.memset(zb[:], 0.0)
    epsb = consts.tile([GPT, 1], F32, name="epsb")
    nc.vector.memset(epsb[:], EPS)

    # ---- PE warm-up: a serial chain of matmuls spanning the DMA/stats front
    # keeps the HAM activity monitor busy so the clock gate opens to 2.4 GHz
    # (~3.4us in) and STAYS open until the first real matmul. Rotates through
    # the pmm ring so it costs no extra PSUM bank.
    warm8 = consts.tile([P, FC], F8, name="warm8")
    nc.vector.memset(warm8[:], 0.0)
    for w in range(NWARM):
        wp = pmm.tile([1, FC], F32, name=f"warm{w}", tag="mm")
        _mm(nc, wp[:], ones_col8[:, 0, 0:1], warm8[:], start=True, stop=True)

    # ---- per-image emission ----
    def new_img(i):
        return {"i": i}

    def emit_load(im):
        i = im["i"]
        im["XB"] = XB_0 if i == 0 else emit_loadx(i, pXB)

    def emit_stats(im):
        i = im["i"]
        XB = im["XB"]
        stats = pS.tile([P, 2 * NCT], F32, name=f"stats{i}", tag="stats")
        scratch = pS.tile([P, HW], DT, name=f"scr{i}", tag="scratch")
        for ct in range(NCT):
            nc.vector.tensor_reduce(stats[:, ct:ct + 1], XB[:, ct, :], AX.X, OP.add)
            nc.scalar.activation(scratch[:], XB[:, ct, :], AF.Square, bias=zb[:],
                                 accum_out=stats[:, NCT + ct:NCT + ct + 1])
        im["stats"] = stats

    def emit_norm(im):
        i = im["i"]
        XB, stats = im["XB"], im["stats"]
        with nc.named_scope(f"norm{i}"):
            gst = paux.tile([GPT, 2 * NCT], F32, name=f"gst{i}", tag="aux")
            _mm(nc, gst[:], gsel[:], stats[:], start=True, stop=True)
            gm = pS.tile([GPT, 2 * NCT], F32, name=f"gm{i}", tag="gm")
            nc.vector.tensor_scalar_mul(gm[:], gst[:], 1.0 / (CPG * HW))
            sq = pS.tile([GPT, NCT], F32, name=f"sq{i}", tag="sq")
            nc.vector.tensor_mul(sq[:], gm[:, 0:NCT], gm[:, 0:NCT])
            var = pS.tile([GPT, NCT], F32, name=f"var{i}", tag="var")
            nc.vector.tensor_sub(var[:], gm[:, NCT:], sq[:])
            # rstd = exp(-0.5*ln(var+eps)) — Ln/Exp live in one ACT table set,
            # unlike Sqrt (whose set swap costs ~2.7us each way)
            lnv = pS.tile([GPT, NCT], F32, name=f"lnv{i}", tag="lnv")
            nc.scalar.activation(lnv[:], var[:], AF.Ln, bias=epsb[:])
            gmr = pS.tile([GPT, 2 * NCT], F32, name=f"gmr{i}", tag="gmr")
            nc.vector.tensor_copy(gmr[:, 0:NCT], gm[:, 0:NCT])
            nc.scalar.activation(gmr[:, NCT:], lnv[:], AF.Exp, bias=zb[0:GPT, :],
                                 scale=-0.5)
            pmr = paux.tile([P, 2 * NCT], F32, name=f"pmr{i}", tag="aux")
            _mm(nc, pmr[:], gselT[:], gmr[:], start=True, stop=True)
            mr = pS.tile([P, 2 * NCT], F32, name=f"mr{i}", tag="mr")
            nc.vector.tensor_copy(mr[:], pmr[:])
            # a = rstd*scale (cols NCT..), b = gn_bias - mean*a (cols 0..NCT)
            ab = pS.tile([P, 2 * NCT], F32, name=f"ab{i}", tag="ab")
            tb = pS.tile([P, NCT], F32, name=f"tb{i}", tag="tb")
            for ct in range(NCT):
                a_col = ab[:, NCT + ct:NCT + ct + 1]
                nc.vector.tensor_mul(a_col, mr[:, NCT + ct:NCT + ct + 1], gs_sb[:, ct:ct + 1])
                nc.vector.tensor_mul(tb[:, ct:ct + 1], mr[:, ct:ct + 1], a_col)
                nc.vector.tensor_sub(ab[:, ct:ct + 1], gb_sb[:, ct:ct + 1], tb[:, ct:ct + 1])
            HN = pHN.tile([P, NCT, HW], F8, name=f"HN{i}", tag="HN")
            for ct in range(NCT):
                nc.vector.tensor_scalar(HN[:, ct, :], XB[:, ct, :],
                                        ab[:, NCT + ct:NCT + ct + 1], ab[:, ct:ct + 1],
                                        OP.mult, OP.add)
            im["HN"] = HN

    def emit_boeff():
        # bo_eff = bo + wo@bv (both commute past the attention average), formed
        # once so the vT eviction needs no bias add. Emitted after norm(b) so
        # its aux-ring slots never gate the groupnorm matmuls.
        boeff = consts.tile([P, NCT], F32, name="boeff")
        for ob in range(NCT):
            ps = paux.tile([P, 1], F32, name=f"wobv{ob}", tag="aux")
            for ct in range(NCT):
                _mm(nc, ps[:], wot_sb[:, ct, ob * P:(ob + 1) * P], bvcol[:, ct:ct + 1],
                    start=(ct == 0), stop=(ct == NCT - 1))
            nc.vector.tensor_add(boeff[:, ob:ob + 1], bo_sb[:, ob:ob + 1], ps[:])
        return boeff

    def emit_qkv(im):
        i = im["i"]
        HN = im["HN"]
        with nc.named_scope(f"qkv{i}"):
            Q = pQ.tile([P, NCT, HW], F8, name=f"Q{i}", tag="Q")
            K = pK.tile([P, NCT, HW], F8, name=f"K{i}", tag="K")
            for wname, bias_sb, OT, on_act in (("wqt", bq_sb, Q, True),
                                               ("wkt", bk_sb, K, False)):
                for ob in range(NCT):
                    ps = pmm.tile([P, NIC, FC], F32, name=f"{wname}ps{i}_{ob}", tag="mm")
                    for ct in range(0, NCT, 2):
                        lhs = w_sb[wname][:, ct:ct + 2, ob * P:(ob + 1) * P]
                        for ic in range(NIC):
                            _mm8(nc, ps[:, ic, :], lhs, HN[:, ct:ct + 2, ic * FC:(ic + 1) * FC],
                                 start=(ct == 0), stop=(ct == NCT - 2))
                    # one [128,1024] eviction per ob; Q on ACT, K on DVE to
                    # balance the two engines' load
                    if on_act:
                        nc.scalar.add(OT[:, ob, :], ps[:], bias_sb[:, ob:ob + 1])
                    else:
                        nc.vector.tensor_scalar_add(OT[:, ob, :], ps[:],
                                                    bias_sb[:, ob:ob + 1])
            VT = pVT.tile([P, NSB, C], F8, name=f"VT{i}", tag="VT")
            for sb in range(0, NSB, 2):
                ps = pmm.tile([P, 2, C], F32, name=f"vtps{i}_{sb}", tag="mm")
                for k in range(2):
                    for ct in range(0, NCT, 2):
                        _mm8(nc, ps[:, k, :], HN[:, ct:ct + 2, (sb + k) * P:(sb + k + 1) * P],
                             w_sb["wvt"][:, ct:ct + 2, 0:C],
                             start=(ct == 0), stop=(ct == NCT - 2))
                nc.vector.tensor_copy(VT[:, sb:sb + 2, :], ps[:])
            im["Q"], im["K"], im["VT"] = Q, K, VT

    def emit_scores(im):
        i = im["i"]
        Q, K = im["Q"], im["K"]
        with nc.named_scope(f"scores{i}"):
            PT = pPT.tile([P, NSB, HW], F8, name=f"PT{i}", tag="PT")
            # den accumulates across jb pairs; its MMs are emitted inside the
            # jb loop so each lands right after the exp that feeds it
            den = paux.tile([1, NIC, FC], F32, name=f"den{i}", tag="aux")
            for jb in range(NSB):
                ps = pmm.tile([P, NIC, FC], F32, name=f"sps{i}_{jb}", tag="mm")
                for ct in range(0, NCT, 2):
                    lhs = K[:, ct:ct + 2, jb * P:(jb + 1) * P]
                    for ic in range(NIC):
                        _mm8(nc, ps[:, ic, :], lhs, Q[:, ct:ct + 2, ic * FC:(ic + 1) * FC],
                             start=(ct == 0), stop=(ct == NCT - 2))
                nc.scalar.activation(PT[:, jb, :], ps[:], AF.Exp, bias=zb[:],
                                     scale=SM_SCALE)
                if jb % 2 == 1:
                    for ic in range(NIC):
                        _mm8(nc, den[:, ic, :], ones_col8[:, 0:2, 0:1],
                             PT[:, jb - 1:jb + 1, ic * FC:(ic + 1) * FC],
                             start=(jb == 1), stop=(jb == NSB - 1))
            lnden = pS.tile([1, HW], DT, name=f"lnden{i}", tag="lnden")
            nc.scalar.activation(lnden[:], den[:], AF.Ln, bias=zb[0:1, :])
            im["PT"], im["lnden"] = PT, lnden

    def emit_attn_num(im):
        i = im["i"]
        VT, PT = im["VT"], im["PT"]
        with nc.named_scope(f"num{i}"):
            # num = vT.T @ P^T with the 1/den softmax normalization folded into
            # the PSUM eviction (commutes with the channel-wise wo projection);
            # 1/den arrives as exp(-lnden) with the broadcast done by a matmul
            # BETWEEN Ln and Exp so no engine touches 1 lane for long. The
            # Ln/rb/Exp chain hides behind the OTHER image's matmuls (qkv(b)
            # for image a, proj(a) for image b).
            recipb = pS.tile([P, HW], F32, name=f"recipb{i}", tag="recipb")
            rb = paux.tile([P, NIC, FC], F32, name=f"rb{i}", tag="aux")
            for ic in range(NIC):
                _mm(nc, rb[:, ic, :], ones_row[:],
                    im["lnden"][:, ic * FC:(ic + 1) * FC], start=True, stop=True)
            nc.scalar.activation(recipb[:], rb[:], AF.Exp, bias=zb[:], scale=-1.0)
            NUM = pNUM.tile([P, NCT, HW], DT, name=f"NUM{i}", tag="NUM")
            for cb in range(NCT):
                ps = pmm.tile([P, NIC, FC], F32, name=f"nps{i}_{cb}", tag="mm")
                for jt in range(0, NSB, 2):
                    lhs = VT[:, jt:jt + 2, cb * P:(cb + 1) * P]
                    for ic in range(NIC):
                        _mm8(nc, ps[:, ic, :], lhs, PT[:, jt:jt + 2, ic * FC:(ic + 1) * FC],
                             start=(jt == 0), stop=(jt == NSB - 2))
                nc.vector.tensor_mul(NUM[:, cb, :], ps[:], recipb[:])
            im["NUM"] = NUM

    def emit_attn_proj(im, boeff):
        i = im["i"]
        XB, NUM = im["XB"], im["NUM"]
        with nc.named_scope(f"proj{i}"):
            # proj + residual (+bo_eff) straight from PSUM, then store; each
            # output block's store gets its own DMA queue
            OUTT = pOUT.tile([P, NCT, HW], F32, name=f"OUT{i}", tag="OUT")
            for ob in range(NCT):
                ps = pmm.tile([P, NIC, FC], F32, name=f"pps{i}_{ob}", tag="mm")
                for ct in range(NCT):
                    lhs = wot_sb[:, ct, ob * P:(ob + 1) * P]
                    for ic in range(NIC):
                        _mm(nc, ps[:, ic, :], lhs, NUM[:, ct, ic * FC:(ic + 1) * FC],
                            start=(ct == 0), stop=(ct == NCT - 1))
                nc.vector.scalar_tensor_tensor(OUTT[:, ob, :], ps[:],
                                               boeff[:, ob:ob + 1], XB[:, ob, :],
                                               OP.add, OP.add)
                qs[ob].dma_start(io["out"][i, ob * P:(ob + 1) * P, :], OUTT[:, ob, :])

    ims = [new_img(i) for i in range(BPC)]
    a, b = ims
    emit_load(a)
    emit_stats(a)
    emit_load(b)
    emit_stats(b)
    emit_norm(a)
    emit_qkv(a)
    emit_norm(b)
    boeff = emit_boeff()
    emit_scores(a)
    emit_qkv(b)          # hides image a's den->ln->recip chain
    emit_attn_num(a)
    emit_scores(b)
    emit_attn_proj(a, boeff)   # hides image b's den->ln->recip chain
    emit_attn_num(b)
    emit_attn_proj(b, boeff)


def _build():
    if "nc" in _CACHE:
        return _CACHE["nc"]
    _pin_act_tables()
    nc = bacc.Bacc("TRN2", target_bir_lowering=False, debug=False, num_devices=NCORES)
    io = {}
    io["x"] = nc.dram_tensor("x", [BPC, C, HW], DT, kind="ExternalInput").ap()
    for wname in ("wqt", "wkt", "wvt"):
        io[wname] = nc.dram_tensor(wname, [P, NCT, C], F8, kind="ExternalInput").ap()
    io["wot"] = nc.dram_tensor("wot", [P, NCT, C], DT, kind="ExternalInput").ap()
    io["cvec"] = nc.dram_tensor("cvec", [P, 5 * NCT + GPT], F32,
                                kind="ExternalInput").ap()
    io["bvcol"] = nc.dram_tensor("bvcol", [P, NCT], DT, kind="ExternalInput").ap()
    io["gselT"] = nc.dram_tensor("gselT", [GPT, P], F32, kind="ExternalInput").ap()
    io["out"] = nc.dram_tensor("out", [BPC, C, HW], F32, kind="ExternalOutput").ap()

    with tile.TileContext(nc) as tc:
        with ExitStack() as ctx:
            _emit(ctx, tc, io)
    nc.compile()
    _CACHE["nc"] = nc
    return nc


def _col_layout(v):
    # (C,) -> (P, NCT): column ct holds channels [ct*128, (ct+1)*128)
    return np.ascontiguousarray(np.asarray(v, np.float32).reshape(NCT, P).T)


def _run(inputs, trace=False, **run_kwargs):
    x = np.ascontiguousarray(np.asarray(inputs["x"], np.float32).reshape(B, C, HW))
    def _wpack(w, scale, npdt):
        # wT (c_in, c_out) -> (P, NCT, C): W[p, ct, j] = wT[ct*128+p, j] * scale
        wt = (np.asarray(w, np.float32).T * scale).astype(npdt)
        return np.ascontiguousarray(wt.reshape(NCT, P, C).transpose(1, 0, 2))

    wdt = {n: _wpack(inputs[s], WS, F8_NP)
           for n, s in (("wqt", "wq"), ("wkt", "wk"), ("wvt", "wv"))}
    wdt["wot"] = _wpack(inputs["wo"], 1.0, DT_NP)
    pidx = np.arange(P)
    gsel = (pidx[:, None] // CPG == np.arange(GPT)[None, :]).astype(np.float32)
    # bq/bk carry the 32x weight scale so Q=32q, K=32k on-device; bv is folded
    # into bo_eff on-device (bo + wo@bv) so vT needs no bias at all
    cvec = np.concatenate([_col_layout(np.asarray(inputs["bq"]) * WS),
                           _col_layout(np.asarray(inputs["bk"]) * WS),
                           _col_layout(inputs["bo"]), _col_layout(inputs["gn_scale"]),
                           _col_layout(inputs["gn_bias"]), gsel], axis=1)
    common = {
        **wdt,
        "cvec": np.ascontiguousarray(cvec),
        "bvcol": np.ascontiguousarray(_col_layout(inputs["bv"]).astype(DT_NP)),
        "gselT": np.ascontiguousarray(gsel.T),
    }
    xb = x.astype(DT_NP)
    in_maps = [{"x": np.ascontiguousarray(xb[m * BPC:(m + 1) * BPC]), **common}
               for m in range(NCORES)]
    nc = _build()
    res = run_bass_kernel_spmd(nc, in_maps, core_ids=list(range(NCORES)),
                               trace=trace, **run_kwargs)
    out = np.concatenate([r["out"] for r in res.results], axis=0)
    return out.reshape(B, C, H, W).astype(np.float32), res


def kernel(**inputs):
    out, _ = _run(inputs)
    return out
